# revision 33
# baseline (speedup 1.0000x reference)
"""Trainium2 Bass kernel for a 2-layer LSTM decoder VAE head.

Strategy: 8-way tensor parallelism over the hidden dim (each core owns 128
rows of each gate / 512 gate rows per layer).  The key discovery from the
v1 trace: with weight-stationary matmuls the kernel is LDWEIGHTS-bound
(~117ns weight load per 53ns 128-wide stream).  So v2 flips the matmul
orientation: the gathered activation tiles (h1/h2, [feature,batch]) are the
*stationary* operand -- each loaded once per 2 weight streams -- and the
weights are the *moving* operand with free dim 512 (213ns streams that hide
the loads).  PSUM layout is batch-major [128b, 512gates]; biases and the
tiny Wih0@x term enter via rank-1/K=9 matmuls (ones-row trick).  The MLP
relu -> x path needs one layout flip, done with 4 small PE transposes per
batch-chunk.

B=256 runs as two 128-wide chunks (M=128 stationary limit) whose cycles
interleave so one chunk's AllGather hides behind the other's matmuls.
"""

import os
import numpy as np

B, SEQ, H, COORD = 256, 200, 1024, 8
LATS = (32, 64, 128)
TOT = sum(LATS)  # 224
N_CORES = 8
HC = H // N_CORES   # 128 rows of h per core
G = 4 * HC          # 512 gate rows per core
KT = H // 128       # 8 K tiles
NB = B // 2         # batch per chunk

_CACHE = {}


def _mmdt():
    return os.environ.get("BASS_KERNEL_MMDT", "bf16")


def _build(seq, mmdt):
    import concourse.bass as bass
    import concourse.tile as tile
    from concourse import bacc, mybir

    f32 = mybir.dt.float32
    DT = {"fp32": mybir.dt.float32, "bf16": mybir.dt.bfloat16}[mmdt]
    AF = mybir.ActivationFunctionType

    nc = bacc.Bacc("TRN2", target_bir_lowering=False, debug=False,
                   num_devices=N_CORES)

    def din(name, shape, dt=None):
        return nc.dram_tensor(name, list(shape), dt or f32,
                              kind="ExternalInput")

    # moving weights: [128 (k within tile), KT, 512 (gate cols)]
    whh0T = din("whh0T", (128, KT, G), DT)
    wih1T = din("wih1T", (128, KT, G), DT)
    whh1T = din("whh1T", (128, KT, G), DT)
    wo1T = din("wo1T", (128, KT, G), DT)       # MLP hidden (replicated)
    wfoldT = din("wfoldT", (128, 4, G), DT)    # (Wih0@Wo2)^T tiles
    wo2T = din("wo2T", (128, 4, COORD), DT)    # lhsT tiles for x
    brows = din("brows", (1, 4, G), DT)        # bias rows: [g1,o1,g0',g0]
    wproj = din("wproj", (128, KT, 2, 128))    # h0 lhsT tiles (f32)
    wprojcT = din("wprojcT", (128, 2, 128))    # c0 moving weights (f32)
    zt = din("zt", (128, 2, B))                # z^T padded, row 224 = ones
    onesr = din("onesr", (1, NB), DT)          # ones row at partition 0
    ident = din("ident", (128, 128), DT)
    bo2 = din("bo2", (COORD, 1))

    OUT = nc.dram_tensor("out", [seq, COORD, B], f32, kind="ExternalOutput")

    def persist(name, shape, dtype=f32):
        return nc.alloc_sbuf_tensor(name, list(shape), dtype).ap()

    whh0T_sb = persist("whh0T_sb", [128, KT, G], DT)
    wih1T_sb = persist("wih1T_sb", [128, KT, G], DT)
    whh1T_sb = persist("whh1T_sb", [128, KT, G], DT)
    wo1T_sb = persist("wo1T_sb", [128, KT, G], DT)
    wfoldT_sb = persist("wfoldT_sb", [128, 4, G], DT)
    wo2T_sb = persist("wo2T_sb", [128, 4, COORD], DT)
    brows_sb = persist("brows_sb", [1, 4, G], DT)
    wproj_sb = persist("wproj_sb", [128, KT, 2, 128])
    wprojcT_sb = persist("wprojcT_sb", [128, 2, 128])
    zt_sb = persist("zt_sb", [128, 2, B])
    ones_sb = persist("ones_sb", [1, NB], DT)
    ident_sb = persist("ident_sb", [128, 128], DT)
    bo2_sb = persist("bo2_sb", [COORD, 1])

    class Chain:
        def __init__(self, name, b0, dmaq):
            self.name = name
            self.b0 = b0
            self.dmaq = dmaq     # dedicated DMA queue engine for this chain
            self.h1T = persist(f"h1T_{name}", [128, KT, NB], DT)
            self.h2T = persist(f"h2T_{name}", [128, KT, NB], DT)
            self.c1 = persist(f"c1_{name}", [128, HC])   # batch-major f32
            self.c2 = persist(f"c2_{name}", [128, HC])
            self.reluT = persist(f"reluT_{name}", [128, 4, HC], DT)
            self.xTf = persist(f"xTf_{name}", [COORD, NB])
            self.g0 = None       # open gates0 psum (batch-major [128, G])
            self.g1_open = None  # g1 psum opened in front, closed in back
            self.pending = {}    # lay -> AllGather output dram tile

    with tile.TileContext(nc) as tc:
        A = Chain("a", 0, nc.sync)
        Bc = Chain("b", NB, nc.scalar)
        chains = (A, Bc)

        for dst, src in (
            (whh0T_sb, whh0T), (wih1T_sb, wih1T), (whh1T_sb, whh1T),
            (wo1T_sb, wo1T), (wfoldT_sb, wfoldT), (wo2T_sb, wo2T),
            (brows_sb, brows), (wproj_sb, wproj), (wprojcT_sb, wprojcT),
            (zt_sb, zt), (ones_sb, onesr), (ident_sb, ident),
            (bo2_sb, bo2),
        ):
            nc.sync.dma_start(dst[:], src.ap())

        with (
            tc.tile_pool(name="ps6", bufs=6, space="PSUM") as ps6,
            tc.tile_pool(name="psS", bufs=2, space="PSUM") as psS,
            tc.tile_pool(name="nl", bufs=4) as nl,
            tc.tile_pool(name="dram", bufs=2, space="DRAM") as dram,
        ):
            # ---- init: h0 (feature-major, replicated) ----
            for m in range(KT):
                ps = psS.tile([128, B], f32, name="ps_h0", tag="s")
                nc.tensor.matmul(ps[:], wproj_sb[:, m, 0, :], zt_sb[:, 0, :],
                                 start=True, stop=False)
                nc.tensor.matmul(ps[:], wproj_sb[:, m, 1, :], zt_sb[:, 1, :],
                                 start=False, stop=True)
                for ch in chains:
                    sl = ps[:, ch.b0:ch.b0 + NB]
                    nc.scalar.activation(ch.h1T[:, m, :], sl, AF.Identity)
                    nc.vector.tensor_copy(ch.h2T[:, m, :], ch.h1T[:, m, :])

            # ---- init: c0 (batch-major per chunk) ----
            for ch in chains:
                ps = psS.tile([128, HC], f32, name="ps_c0", tag="s")
                for kk in range(2):
                    nc.tensor.matmul(ps[:], zt_sb[:, kk, ch.b0:ch.b0 + NB],
                                     wprojcT_sb[:, kk, :],
                                     start=(kk == 0), stop=(kk == 1))
                nc.scalar.activation(ch.c1[:, :], ps[:], AF.Identity)
                nc.vector.tensor_copy(ch.c2[:, :], ch.c1[:, :])

            # ---- init: open gates0(0) = Whh0 @ h0 ----
            for ch in chains:
                g0 = ps6.tile([128, G], f32, name="ps_g0", tag="g")
                for k in range(KT):
                    nc.tensor.matmul(g0[:], ch.h1T[:, k, :], whh0T_sb[:, k, :],
                                     start=(k == 0), stop=False)
                ch.g0 = g0

            def gather(ch, src_sb, lay):
                """Launch AllGather of the transposed h chunk (SBUF
                [128f, NB]).  The scatter back to SBUF is emitted separately
                at the consumption point (scatter())."""
                sfx = f"{ch.name}{lay}"
                inb = dram.tile([128, NB], DT, name="agin", tag=f"agi_{sfx}")
                outb = dram.tile([128 * N_CORES, NB], DT, name="agout",
                                 tag=f"ago_{sfx}")
                nc.sync.dma_start(inb[:], src_sb[:])
                nc.gpsimd.collective_compute(
                    "AllGather", mybir.AluOpType.bypass,
                    replica_groups=[list(range(N_CORES))],
                    ins=[inb.opt()], outs=[outb.opt()],
                )
                ch.pending[lay] = outb

            def scatter(ch, lay, dest):
                outb = ch.pending[lay]
                nc.sync.dma_start(
                    dest[:, :, :],
                    outb[:, :].rearrange("(k p) n -> p k n", p=128))

            def nonlin(ch, gps, c_sb, lay):
                """gps: batch-major psum [128b, 512] = [i|f|g|o].
                Returns h chunk [128b, 128f] DT in SBUF."""
                sfx = f"{ch.name}{lay}"
                sig_if = nl.tile([128, 2 * HC], f32, name="sig_if",
                                 tag=f"si_{sfx}")
                tan_g = nl.tile([128, HC], f32, name="tan_g", tag=f"tg_{sfx}")
                sig_o = nl.tile([128, HC], f32, name="sig_o", tag=f"so_{sfx}")
                nc.scalar.activation(sig_if[:], gps[:, 0:2 * HC], AF.Sigmoid)
                nc.scalar.activation(tan_g[:], gps[:, 2 * HC:3 * HC], AF.Tanh)
                nc.scalar.activation(sig_o[:], gps[:, 3 * HC:4 * HC],
                                     AF.Sigmoid)
                t_fc = nl.tile([128, HC], f32, name="t_fc", tag=f"fc_{sfx}")
                t_ig = nl.tile([128, HC], f32, name="t_ig", tag=f"ig_{sfx}")
                nc.vector.tensor_mul(t_fc[:], sig_if[:, HC:2 * HC], c_sb[:, :])
                nc.vector.tensor_mul(t_ig[:], sig_if[:, 0:HC], tan_g[:])
                nc.vector.tensor_add(c_sb[:, :], t_fc[:], t_ig[:])
                tan_c = nl.tile([128, HC], f32, name="tan_c", tag=f"tc_{sfx}")
                nc.scalar.activation(tan_c[:], c_sb[:, :], AF.Tanh)
                hch = nl.tile([128, HC], DT, name="hch", tag=f"h_{sfx}")
                nc.vector.tensor_mul(hch[:], sig_o[:], tan_c[:])
                return hch

            def transpose_gather(ch, hch, lay):
                tp = psS.tile([128, NB], DT, name="ps_tr", tag="s")
                nc.tensor.transpose(tp[:], hch[:], ident_sb[:])
                hT = nl.tile([128, NB], DT, name="hT",
                             tag=f"hT_{ch.name}{lay}")
                nc.vector.tensor_copy(hT[:], tp[:])
                gather(ch, hT, lay)

            def emit_front(ch, t):
                """Needs h2T = h2(t-1) gathered.  h2-stationary matmul block;
                (t>0) relu(t-1) + transposes + the Wfold term completing
                gates0(t); nonlin0 -> h1(t); launch AG1(t); then the
                off-cycle x(t-1) -> OUT store (fills the AG wait)."""
                if t > 0:
                    scatter(ch, 1, ch.h2T)   # AG2(t-1) lands here
                g1 = ps6.tile([128, G], f32, name="ps_g1", tag="g")
                ch.g1_open = g1
                mlp = None
                if t > 0:
                    mlp = ps6.tile([128, G], f32, name="ps_mlp", tag="g")
                for k in range(KT):
                    nc.tensor.matmul(g1[:], ch.h2T[:, k, :],
                                     whh1T_sb[:, k, :],
                                     start=(k == 0), stop=False)
                    if t > 0:
                        nc.tensor.matmul(mlp[:], ch.h2T[:, k, :],
                                         wo1T_sb[:, k, :],
                                         start=(k == 0), stop=False)
                nc.tensor.matmul(g1[:], ones_sb[:], brows_sb[0:1, 0, :],
                                 start=False, stop=False)
                if t > 0:
                    nc.tensor.matmul(mlp[:], ones_sb[:], brows_sb[0:1, 1, :],
                                     start=False, stop=True)
                    relu = nl.tile([128, G], DT, name="relu",
                                   tag=f"relu_{ch.name}")
                    nc.scalar.activation(relu[:], mlp[:], AF.Relu)
                    for j in range(4):
                        tp = psS.tile([128, HC], DT, name="ps_rT", tag="s")
                        nc.tensor.transpose(
                            tp[:], relu[:, j * HC:(j + 1) * HC], ident_sb[:])
                        nc.vector.tensor_copy(ch.reluT[:, j, :], tp[:])
                    # gates0(t) += Wfold @ relu(t-1) + bias' (folds Wih0@x)
                    nc.tensor.matmul(ch.g0[:], ones_sb[:],
                                     brows_sb[0:1, 2, :],
                                     start=False, stop=False)
                    for j in range(4):
                        nc.tensor.matmul(ch.g0[:], ch.reluT[:, j, :],
                                         wfoldT_sb[:, j, :],
                                         start=False, stop=(j == 3))
                else:
                    nc.tensor.matmul(ch.g0[:], ones_sb[:],
                                     brows_sb[0:1, 3, :],
                                     start=False, stop=True)
                h1ch = nonlin(ch, ch.g0, ch.c1, 0)
                transpose_gather(ch, h1ch, 0)
                if t > 0:
                    # off-cycle: x(t-1) for the OUT store, during the AG
                    psx = psS.tile([COORD, NB], f32, name="ps_x", tag="s")
                    for j in range(4):
                        nc.tensor.matmul(psx[:], wo2T_sb[:, j, :],
                                         ch.reluT[:, j, :],
                                         start=(j == 0), stop=(j == 3))
                    nc.scalar.activation(ch.xTf[:, :], psx[:],
                                         AF.Identity, bias=bo2_sb[:, 0:1])
                    nc.scalar.dma_start(OUT.ap()[t - 1][:, ch.b0:ch.b0 + NB],
                                        ch.xTf[:, :])

            def emit_back(ch, t):
                """Needs h1T = h1(t) gathered.  Completes gates1(t) (Wih1
                part), opens gates0(t+1) (Whh0 part); nonlin1 -> h2(t);
                launches AG2(t)."""
                scatter(ch, 0, ch.h1T)   # AG1(t) lands here
                last = t == seq - 1
                g1 = ch.g1_open
                g0n = None
                if not last:
                    g0n = ps6.tile([128, G], f32, name="ps_g0", tag="g")
                for k in range(KT):
                    nc.tensor.matmul(g1[:], ch.h1T[:, k, :],
                                     wih1T_sb[:, k, :],
                                     start=False, stop=(k == KT - 1))
                    if not last:
                        nc.tensor.matmul(g0n[:], ch.h1T[:, k, :],
                                         whh0T_sb[:, k, :],
                                         start=(k == 0), stop=False)
                ch.g0 = g0n
                h2ch = nonlin(ch, g1, ch.c2, 1)
                transpose_gather(ch, h2ch, 1)

            def emit_tail(ch):
                """x(seq-1) from gathered h2(seq-1): MLP only, store OUT."""
                scatter(ch, 1, ch.h2T)   # AG2(seq-1) lands here
                mlp = ps6.tile([128, G], f32, name="ps_mlp", tag="g")
                for k in range(KT):
                    nc.tensor.matmul(mlp[:], ch.h2T[:, k, :],
                                     wo1T_sb[:, k, :],
                                     start=(k == 0), stop=False)
                nc.tensor.matmul(mlp[:], ones_sb[:], brows_sb[0:1, 1, :],
                                 start=False, stop=True)
                relu = nl.tile([128, G], DT, name="relu",
                               tag=f"relu_{ch.name}")
                nc.scalar.activation(relu[:], mlp[:], AF.Relu)
                for j in range(4):
                    tp = psS.tile([128, HC], DT, name="ps_rT", tag="s")
                    nc.tensor.transpose(
                        tp[:], relu[:, j * HC:(j + 1) * HC], ident_sb[:])
                    nc.vector.tensor_copy(ch.reluT[:, j, :], tp[:])
                psx = psS.tile([COORD, NB], f32, name="ps_x", tag="s")
                for j in range(4):
                    nc.tensor.matmul(psx[:], wo2T_sb[:, j, :],
                                     ch.reluT[:, j, :],
                                     start=(j == 0), stop=(j == 3))
                nc.scalar.activation(ch.xTf[:, :], psx[:], AF.Identity,
                                     bias=bo2_sb[:, 0:1])
                nc.scalar.dma_start(OUT.ap()[seq - 1][:, ch.b0:ch.b0 + NB],
                                    ch.xTf[:, :])

            for t in range(seq):
                for ch in chains:
                    emit_front(ch, t)
                for ch in chains:
                    emit_back(ch, t)
            for ch in chains:
                emit_tail(ch)

    nc.compile()
    return nc


def _prep_inputs(inputs):
    import ml_dtypes
    np_dt = {"fp32": np.float32, "bf16": ml_dtypes.bfloat16}[_mmdt()]
    f = lambda k: np.asarray(inputs[k], np.float32)
    W_proj, b_proj = f("W_proj"), f("b_proj")
    W_ih0, W_hh0 = f("W_ih0"), f("W_hh0")
    b_ih0, b_hh0 = f("b_ih0"), f("b_hh0")
    W_ih1, W_hh1 = f("W_ih1"), f("W_hh1")
    b_ih1, b_hh1 = f("b_ih1"), f("b_hh1")
    W_o1, b_o1 = f("W_o1"), f("b_o1")
    W_o2, b_o2 = f("W_o2"), f("b_o2")
    z = np.concatenate([f("z_primitive"), f("z_skill"), f("z_style")], axis=1)

    bias_g0 = b_ih0 + b_hh0
    bias_g1 = b_ih1 + b_hh1

    # z^T padded to 256 rows, row 224 = ones (bias row for the init matmuls)
    ztp = np.zeros((256, B), np.float32)
    ztp[:TOT] = z.T
    ztp[TOT] = 1.0
    zt = np.ascontiguousarray(ztp.reshape(2, 128, B).transpose(1, 0, 2))

    ident = np.eye(128, dtype=np.float32).astype(np_dt)
    onesr = np.ones((1, NB), np.float32).astype(np_dt)

    def movingT(W, rows):
        """W[rows] -> moving rhs tiles [128, KT', len(rows)]:
        out[p, k, n] = W[rows[n], k*128+p]."""
        Wt = np.ascontiguousarray(W[rows].T)  # [K, G]
        K = Wt.shape[0]
        return np.ascontiguousarray(
            Wt.reshape(K // 128, 128, len(rows)).transpose(1, 0, 2)
        ).astype(np_dt)

    # MLP hidden weights (replicated): K=1024 over h2, N=512 hidden
    wo1T = movingT(W_o1, np.arange(H // 2))
    # x lhsT tiles: wo2T[p, j, m] = W_o2[m, j*128+p]
    wo2T = np.ascontiguousarray(
        W_o2.T.reshape(4, 128, COORD).transpose(1, 0, 2)).astype(np_dt)
    bo2_col = b_o2.reshape(COORD, 1).astype(np.float32)

    # h0 lhsT tiles (f32): wproj[p, m, kk, n] = Wp[m*128+n, kk*128+p]
    Wp = np.zeros((H, 256), np.float32)
    Wp[:, :TOT] = W_proj[:H]
    Wp[:, TOT] = b_proj[:H]
    wproj = np.ascontiguousarray(
        Wp.reshape(KT, 128, 2, 128).transpose(3, 0, 2, 1))

    in_maps = []
    for c in range(N_CORES):
        rows_g = np.concatenate(
            [g * H + c * HC + np.arange(HC) for g in range(4)])
        # c0 moving weights: wprojcT[p, kk, n] = Wpc[n, kk*128+p]
        own = H + c * HC + np.arange(HC)
        Wpc = np.zeros((HC, 256), np.float32)
        Wpc[:, :TOT] = W_proj[own]
        Wpc[:, TOT] = b_proj[own]
        wprojcT = np.ascontiguousarray(
            Wpc.T.reshape(2, 128, HC).transpose(1, 0, 2))

        whh0T = movingT(W_hh0, rows_g)
        wih1T = movingT(W_ih1, rows_g)
        whh1T = movingT(W_hh1, rows_g)

        # Wfold = Wih0[rows_g] @ W_o2: folds the x-term into gates0
        Wfold = W_ih0[rows_g] @ W_o2          # [G, 512]
        wfoldT = np.ascontiguousarray(
            Wfold.T.reshape(4, 128, G).transpose(1, 0, 2)).astype(np_dt)

        brows = np.zeros((1, 4, G), np.float32)
        brows[0, 0] = bias_g1[rows_g]
        brows[0, 1] = b_o1
        brows[0, 2] = bias_g0[rows_g] + W_ih0[rows_g] @ b_o2
        brows[0, 3] = bias_g0[rows_g]

        in_maps.append({
            "whh0T": whh0T, "wih1T": wih1T, "whh1T": whh1T,
            "wo1T": wo1T,
            "wfoldT": wfoldT,
            "wo2T": wo2T,
            "brows": brows.astype(np_dt),
            "wproj": wproj,
            "wprojcT": wprojcT,
            "zt": zt,
            "onesr": onesr,
            "ident": ident,
            "bo2": bo2_col,
        })
    return in_maps


def kernel(**inputs):
    from concourse.bass_utils import run_bass_kernel_spmd

    seq = int(os.environ.get("BASS_KERNEL_SEQ", SEQ))
    key = (seq, _mmdt())
    if key not in _CACHE:
        _CACHE[key] = _build(seq, _mmdt())
    nc = _CACHE[key]
    in_maps = _prep_inputs(inputs)

    trace = os.environ.get("BASS_KERNEL_TRACE", "") == "1"
    kwargs = {}
    if trace:
        kwargs["trace"] = True
        kwargs["tmpdir"] = os.environ.get("BASS_KERNEL_TRACE_DIR") or None
    res = run_bass_kernel_spmd(nc, in_maps, core_ids=list(range(N_CORES)),
                               **kwargs)
    if trace:
        kernel.last_exec_time_ns = res.exec_time_ns
    out = res.results[0]["out"]          # [seq, 8, B]
    return np.ascontiguousarray(out.transpose(2, 0, 1)).astype(np.float32)


kernel.last_exec_time_ns = None


# revision 34
# speedup vs baseline: 1.1840x; 1.1840x over previous
"""Trainium2 Bass kernel for a 2-layer LSTM decoder VAE head.

Strategy: 8-way tensor parallelism over the hidden dim (each core owns 128
rows of each gate / 512 gate rows per layer).  The key discovery from the
v1 trace: with weight-stationary matmuls the kernel is LDWEIGHTS-bound
(~117ns weight load per 53ns 128-wide stream).  So v2 flips the matmul
orientation: the gathered activation tiles (h1/h2, [feature,batch]) are the
*stationary* operand -- each loaded once per 2 weight streams -- and the
weights are the *moving* operand with free dim 512 (213ns streams that hide
the loads).  PSUM layout is batch-major [128b, 512gates]; biases and the
tiny Wih0@x term enter via rank-1/K=9 matmuls (ones-row trick).  The MLP
relu -> x path needs one layout flip, done with 4 small PE transposes per
batch-chunk.

B=256 runs as two 128-wide chunks (M=128 stationary limit) whose cycles
interleave so one chunk's AllGather hides behind the other's matmuls.
"""

import os
import numpy as np

B, SEQ, H, COORD = 256, 200, 1024, 8
LATS = (32, 64, 128)
TOT = sum(LATS)  # 224
N_CORES = 8
HC = H // N_CORES   # 128 rows of h per core
G = 4 * HC          # 512 gate rows per core
KT = H // 128       # 8 K tiles
NB = B // 2         # batch per chunk

_CACHE = {}


def _mmdt():
    return os.environ.get("BASS_KERNEL_MMDT", "bf16")


def _build(seq, mmdt):
    import concourse.bass as bass
    import concourse.tile as tile
    from concourse import bacc, mybir

    f32 = mybir.dt.float32
    DT = {"fp32": mybir.dt.float32, "bf16": mybir.dt.bfloat16}[mmdt]
    AF = mybir.ActivationFunctionType

    nc = bacc.Bacc("TRN2", target_bir_lowering=False, debug=False,
                   num_devices=N_CORES)

    def din(name, shape, dt=None):
        return nc.dram_tensor(name, list(shape), dt or f32,
                              kind="ExternalInput")

    # moving weights: [128 (k within tile), KT, 512 (gate cols)]
    whh0T = din("whh0T", (128, KT, G), DT)
    wih1T = din("wih1T", (128, KT, G), DT)
    whh1T = din("whh1T", (128, KT, G), DT)
    wo1T = din("wo1T", (128, KT, G), DT)       # MLP hidden (replicated)
    wfoldT = din("wfoldT", (128, 4, G), DT)    # (Wih0@Wo2)^T tiles
    wo2T = din("wo2T", (128, 4, COORD), DT)    # lhsT tiles for x
    brows = din("brows", (1, 4, G), DT)        # bias rows: [g1,o1,g0',g0]
    wproj = din("wproj", (128, KT, 2, 128))    # h0 lhsT tiles (f32)
    wprojcT = din("wprojcT", (128, 2, 128))    # c0 moving weights (f32)
    zt = din("zt", (128, 2, B))                # z^T padded, row 224 = ones
    onesr = din("onesr", (1, NB), DT)          # ones row at partition 0
    ident = din("ident", (128, 128), DT)
    bo2 = din("bo2", (COORD, 1))

    OUT = nc.dram_tensor("out", [seq, COORD, B], f32, kind="ExternalOutput")

    def persist(name, shape, dtype=f32):
        return nc.alloc_sbuf_tensor(name, list(shape), dtype).ap()

    whh0T_sb = persist("whh0T_sb", [128, KT, G], DT)
    wih1T_sb = persist("wih1T_sb", [128, KT, G], DT)
    whh1T_sb = persist("whh1T_sb", [128, KT, G], DT)
    wo1T_sb = persist("wo1T_sb", [128, KT, G], DT)
    wfoldT_sb = persist("wfoldT_sb", [128, 4, G], DT)
    wo2T_sb = persist("wo2T_sb", [128, 4, COORD], DT)
    brows_sb = persist("brows_sb", [1, 4, G], DT)
    wproj_sb = persist("wproj_sb", [128, KT, 2, 128])
    wprojcT_sb = persist("wprojcT_sb", [128, 2, 128])
    zt_sb = persist("zt_sb", [128, 2, B])
    ones_sb = persist("ones_sb", [1, NB], DT)
    ident_sb = persist("ident_sb", [128, 128], DT)
    bo2_sb = persist("bo2_sb", [COORD, 1])

    class Chain:
        def __init__(self, name, b0, dmaq):
            self.name = name
            self.b0 = b0
            self.dmaq = dmaq     # dedicated DMA queue engine for this chain
            self.h1T = persist(f"h1T_{name}", [128, KT, NB], DT)
            self.h2T = persist(f"h2T_{name}", [128, KT, NB], DT)
            self.c1 = persist(f"c1_{name}", [128, HC])   # batch-major f32
            self.c2 = persist(f"c2_{name}", [128, HC])
            self.reluT = persist(f"reluT_{name}", [128, 4, HC], DT)
            self.xTf = persist(f"xTf_{name}", [COORD, NB])
            self.g0 = None       # open gates0 psum (batch-major [128, G])
            self.g1_open = None  # g1 psum opened in front, closed in back
            self.pending = {}    # lay -> AllGather output dram tile

    with tile.TileContext(nc) as tc:
        A = Chain("a", 0, nc.sync)
        Bc = Chain("b", NB, nc.scalar)
        chains = (A, Bc)

        for dst, src in (
            (whh0T_sb, whh0T), (wih1T_sb, wih1T), (whh1T_sb, whh1T),
            (wo1T_sb, wo1T), (wfoldT_sb, wfoldT), (wo2T_sb, wo2T),
            (brows_sb, brows), (wproj_sb, wproj), (wprojcT_sb, wprojcT),
            (zt_sb, zt), (ones_sb, onesr), (ident_sb, ident),
            (bo2_sb, bo2),
        ):
            nc.sync.dma_start(dst[:], src.ap())

        with (
            tc.tile_pool(name="ps6", bufs=6, space="PSUM") as ps6,
            tc.tile_pool(name="psS", bufs=2, space="PSUM") as psS,
            tc.tile_pool(name="nl", bufs=4) as nl,
            tc.tile_pool(name="dram", bufs=2, space="DRAM") as dram,
        ):
            # ---- init: h0 (feature-major, replicated) ----
            for m in range(KT):
                ps = psS.tile([128, B], f32, name="ps_h0", tag="s")
                nc.tensor.matmul(ps[:], wproj_sb[:, m, 0, :], zt_sb[:, 0, :],
                                 start=True, stop=False)
                nc.tensor.matmul(ps[:], wproj_sb[:, m, 1, :], zt_sb[:, 1, :],
                                 start=False, stop=True)
                for ch in chains:
                    sl = ps[:, ch.b0:ch.b0 + NB]
                    nc.scalar.activation(ch.h1T[:, m, :], sl, AF.Identity)
                    nc.vector.tensor_copy(ch.h2T[:, m, :], ch.h1T[:, m, :])

            # ---- init: c0 (batch-major per chunk) ----
            for ch in chains:
                ps = psS.tile([128, HC], f32, name="ps_c0", tag="s")
                for kk in range(2):
                    nc.tensor.matmul(ps[:], zt_sb[:, kk, ch.b0:ch.b0 + NB],
                                     wprojcT_sb[:, kk, :],
                                     start=(kk == 0), stop=(kk == 1))
                nc.scalar.activation(ch.c1[:, :], ps[:], AF.Identity)
                nc.vector.tensor_copy(ch.c2[:, :], ch.c1[:, :])

            # ---- init: open gates0(0) = Whh0 @ h0 ----
            for ch in chains:
                g0 = ps6.tile([128, G], f32, name="ps_g0", tag="g")
                for k in range(KT):
                    nc.tensor.matmul(g0[:], ch.h1T[:, k, :], whh0T_sb[:, k, :],
                                     start=(k == 0), stop=False)
                ch.g0 = g0

            def gather(ch, src_sb, lay):
                """Launch AllGather of the transposed h chunk (SBUF
                [128f, NB]).  The scatter back to SBUF is emitted separately
                at the consumption point (scatter())."""
                sfx = f"{ch.name}{lay}"
                inb = dram.tile([128, NB], DT, name="agin", tag=f"agi_{sfx}")
                outb = dram.tile([128 * N_CORES, NB], DT, name="agout",
                                 tag=f"ago_{sfx}")
                nc.scalar.dma_start(inb[:], src_sb[:])
                nc.gpsimd.collective_compute(
                    "AllGather", mybir.AluOpType.bypass,
                    replica_groups=[list(range(N_CORES))],
                    ins=[inb.opt()], outs=[outb.opt()],
                )
                ch.pending[lay] = outb

            def scatter(ch, lay, dest):
                outb = ch.pending[lay]
                nc.sync.dma_start(
                    dest[:, :, :],
                    outb[:, :].rearrange("(k p) n -> p k n", p=128))

            def nonlin(ch, gps, c_sb, lay):
                """gps: batch-major psum [128b, 512] = [i|f|g|o].
                Returns h chunk [128b, 128f] DT in SBUF."""
                sfx = f"{ch.name}{lay}"
                sig_if = nl.tile([128, 2 * HC], f32, name="sig_if",
                                 tag=f"si_{sfx}")
                tan_g = nl.tile([128, HC], f32, name="tan_g", tag=f"tg_{sfx}")
                sig_o = nl.tile([128, HC], f32, name="sig_o", tag=f"so_{sfx}")
                nc.scalar.activation(sig_if[:], gps[:, 0:2 * HC], AF.Sigmoid)
                nc.scalar.activation(tan_g[:], gps[:, 2 * HC:3 * HC], AF.Tanh)
                nc.scalar.activation(sig_o[:], gps[:, 3 * HC:4 * HC],
                                     AF.Sigmoid)
                t_fc = nl.tile([128, HC], f32, name="t_fc", tag=f"fc_{sfx}")
                t_ig = nl.tile([128, HC], f32, name="t_ig", tag=f"ig_{sfx}")
                nc.vector.tensor_mul(t_fc[:], sig_if[:, HC:2 * HC], c_sb[:, :])
                nc.vector.tensor_mul(t_ig[:], sig_if[:, 0:HC], tan_g[:])
                nc.vector.tensor_add(c_sb[:, :], t_fc[:], t_ig[:])
                tan_c = nl.tile([128, HC], f32, name="tan_c", tag=f"tc_{sfx}")
                nc.scalar.activation(tan_c[:], c_sb[:, :], AF.Tanh)
                hch = nl.tile([128, HC], DT, name="hch", tag=f"h_{sfx}")
                nc.vector.tensor_mul(hch[:], sig_o[:], tan_c[:])
                return hch

            def transpose_gather(ch, hch, lay):
                tp = psS.tile([128, NB], DT, name="ps_tr", tag="s")
                nc.tensor.transpose(tp[:], hch[:], ident_sb[:])
                hT = nl.tile([128, NB], DT, name="hT",
                             tag=f"hT_{ch.name}{lay}")
                nc.vector.tensor_copy(hT[:], tp[:])
                gather(ch, hT, lay)

            def emit_front(ch, t):
                """Needs h2T = h2(t-1) gathered.  h2-stationary matmul block;
                (t>0) relu(t-1) + transposes + the Wfold term completing
                gates0(t); nonlin0 -> h1(t); launch AG1(t); then the
                off-cycle x(t-1) -> OUT store (fills the AG wait)."""
                if t > 0:
                    scatter(ch, 1, ch.h2T)   # AG2(t-1) lands here
                g1 = ps6.tile([128, G], f32, name="ps_g1", tag="g")
                ch.g1_open = g1
                mlp = None
                if t > 0:
                    mlp = ps6.tile([128, G], f32, name="ps_mlp", tag="g")
                for k in range(KT):
                    nc.tensor.matmul(g1[:], ch.h2T[:, k, :],
                                     whh1T_sb[:, k, :],
                                     start=(k == 0), stop=False)
                    if t > 0:
                        nc.tensor.matmul(mlp[:], ch.h2T[:, k, :],
                                         wo1T_sb[:, k, :],
                                         start=(k == 0), stop=False)
                nc.tensor.matmul(g1[:], ones_sb[:], brows_sb[0:1, 0, :],
                                 start=False, stop=False)
                if t > 0:
                    nc.tensor.matmul(mlp[:], ones_sb[:], brows_sb[0:1, 1, :],
                                     start=False, stop=True)
                    relu = nl.tile([128, G], DT, name="relu",
                                   tag=f"relu_{ch.name}")
                    nc.scalar.activation(relu[:], mlp[:], AF.Relu)
                    for j in range(4):
                        tp = psS.tile([128, HC], DT, name="ps_rT", tag="s")
                        nc.tensor.transpose(
                            tp[:], relu[:, j * HC:(j + 1) * HC], ident_sb[:])
                        nc.vector.tensor_copy(ch.reluT[:, j, :], tp[:])
                    # gates0(t) += Wfold @ relu(t-1) + bias' (folds Wih0@x)
                    nc.tensor.matmul(ch.g0[:], ones_sb[:],
                                     brows_sb[0:1, 2, :],
                                     start=False, stop=False)
                    for j in range(4):
                        nc.tensor.matmul(ch.g0[:], ch.reluT[:, j, :],
                                         wfoldT_sb[:, j, :],
                                         start=False, stop=(j == 3))
                else:
                    nc.tensor.matmul(ch.g0[:], ones_sb[:],
                                     brows_sb[0:1, 3, :],
                                     start=False, stop=True)
                h1ch = nonlin(ch, ch.g0, ch.c1, 0)
                transpose_gather(ch, h1ch, 0)
                if t > 0:
                    # off-cycle: x(t-1) for the OUT store, during the AG
                    psx = psS.tile([COORD, NB], f32, name="ps_x", tag="s")
                    for j in range(4):
                        nc.tensor.matmul(psx[:], wo2T_sb[:, j, :],
                                         ch.reluT[:, j, :],
                                         start=(j == 0), stop=(j == 3))
                    nc.scalar.activation(ch.xTf[:, :], psx[:],
                                         AF.Identity, bias=bo2_sb[:, 0:1])
                    nc.scalar.dma_start(OUT.ap()[t - 1][:, ch.b0:ch.b0 + NB],
                                        ch.xTf[:, :])

            def emit_back(ch, t):
                """Needs h1T = h1(t) gathered.  Completes gates1(t) (Wih1
                part), opens gates0(t+1) (Whh0 part); nonlin1 -> h2(t);
                launches AG2(t)."""
                scatter(ch, 0, ch.h1T)   # AG1(t) lands here
                last = t == seq - 1
                g1 = ch.g1_open
                g0n = None
                if not last:
                    g0n = ps6.tile([128, G], f32, name="ps_g0", tag="g")
                for k in range(KT):
                    nc.tensor.matmul(g1[:], ch.h1T[:, k, :],
                                     wih1T_sb[:, k, :],
                                     start=False, stop=(k == KT - 1))
                    if not last:
                        nc.tensor.matmul(g0n[:], ch.h1T[:, k, :],
                                         whh0T_sb[:, k, :],
                                         start=(k == 0), stop=False)
                ch.g0 = g0n
                h2ch = nonlin(ch, g1, ch.c2, 1)
                transpose_gather(ch, h2ch, 1)

            def emit_tail(ch):
                """x(seq-1) from gathered h2(seq-1): MLP only, store OUT."""
                scatter(ch, 1, ch.h2T)   # AG2(seq-1) lands here
                mlp = ps6.tile([128, G], f32, name="ps_mlp", tag="g")
                for k in range(KT):
                    nc.tensor.matmul(mlp[:], ch.h2T[:, k, :],
                                     wo1T_sb[:, k, :],
                                     start=(k == 0), stop=False)
                nc.tensor.matmul(mlp[:], ones_sb[:], brows_sb[0:1, 1, :],
                                 start=False, stop=True)
                relu = nl.tile([128, G], DT, name="relu",
                               tag=f"relu_{ch.name}")
                nc.scalar.activation(relu[:], mlp[:], AF.Relu)
                for j in range(4):
                    tp = psS.tile([128, HC], DT, name="ps_rT", tag="s")
                    nc.tensor.transpose(
                        tp[:], relu[:, j * HC:(j + 1) * HC], ident_sb[:])
                    nc.vector.tensor_copy(ch.reluT[:, j, :], tp[:])
                psx = psS.tile([COORD, NB], f32, name="ps_x", tag="s")
                for j in range(4):
                    nc.tensor.matmul(psx[:], wo2T_sb[:, j, :],
                                     ch.reluT[:, j, :],
                                     start=(j == 0), stop=(j == 3))
                nc.scalar.activation(ch.xTf[:, :], psx[:], AF.Identity,
                                     bias=bo2_sb[:, 0:1])
                nc.scalar.dma_start(OUT.ap()[seq - 1][:, ch.b0:ch.b0 + NB],
                                    ch.xTf[:, :])

            for t in range(seq):
                for ch in chains:
                    emit_front(ch, t)
                for ch in chains:
                    emit_back(ch, t)
            for ch in chains:
                emit_tail(ch)

    nc.compile()
    return nc


def _prep_inputs(inputs):
    import ml_dtypes
    np_dt = {"fp32": np.float32, "bf16": ml_dtypes.bfloat16}[_mmdt()]
    f = lambda k: np.asarray(inputs[k], np.float32)
    W_proj, b_proj = f("W_proj"), f("b_proj")
    W_ih0, W_hh0 = f("W_ih0"), f("W_hh0")
    b_ih0, b_hh0 = f("b_ih0"), f("b_hh0")
    W_ih1, W_hh1 = f("W_ih1"), f("W_hh1")
    b_ih1, b_hh1 = f("b_ih1"), f("b_hh1")
    W_o1, b_o1 = f("W_o1"), f("b_o1")
    W_o2, b_o2 = f("W_o2"), f("b_o2")
    z = np.concatenate([f("z_primitive"), f("z_skill"), f("z_style")], axis=1)

    bias_g0 = b_ih0 + b_hh0
    bias_g1 = b_ih1 + b_hh1

    # z^T padded to 256 rows, row 224 = ones (bias row for the init matmuls)
    ztp = np.zeros((256, B), np.float32)
    ztp[:TOT] = z.T
    ztp[TOT] = 1.0
    zt = np.ascontiguousarray(ztp.reshape(2, 128, B).transpose(1, 0, 2))

    ident = np.eye(128, dtype=np.float32).astype(np_dt)
    onesr = np.ones((1, NB), np.float32).astype(np_dt)

    def movingT(W, rows):
        """W[rows] -> moving rhs tiles [128, KT', len(rows)]:
        out[p, k, n] = W[rows[n], k*128+p]."""
        Wt = np.ascontiguousarray(W[rows].T)  # [K, G]
        K = Wt.shape[0]
        return np.ascontiguousarray(
            Wt.reshape(K // 128, 128, len(rows)).transpose(1, 0, 2)
        ).astype(np_dt)

    # MLP hidden weights (replicated): K=1024 over h2, N=512 hidden
    wo1T = movingT(W_o1, np.arange(H // 2))
    # x lhsT tiles: wo2T[p, j, m] = W_o2[m, j*128+p]
    wo2T = np.ascontiguousarray(
        W_o2.T.reshape(4, 128, COORD).transpose(1, 0, 2)).astype(np_dt)
    bo2_col = b_o2.reshape(COORD, 1).astype(np.float32)

    # h0 lhsT tiles (f32): wproj[p, m, kk, n] = Wp[m*128+n, kk*128+p]
    Wp = np.zeros((H, 256), np.float32)
    Wp[:, :TOT] = W_proj[:H]
    Wp[:, TOT] = b_proj[:H]
    wproj = np.ascontiguousarray(
        Wp.reshape(KT, 128, 2, 128).transpose(3, 0, 2, 1))

    in_maps = []
    for c in range(N_CORES):
        rows_g = np.concatenate(
            [g * H + c * HC + np.arange(HC) for g in range(4)])
        # c0 moving weights: wprojcT[p, kk, n] = Wpc[n, kk*128+p]
        own = H + c * HC + np.arange(HC)
        Wpc = np.zeros((HC, 256), np.float32)
        Wpc[:, :TOT] = W_proj[own]
        Wpc[:, TOT] = b_proj[own]
        wprojcT = np.ascontiguousarray(
            Wpc.T.reshape(2, 128, HC).transpose(1, 0, 2))

        whh0T = movingT(W_hh0, rows_g)
        wih1T = movingT(W_ih1, rows_g)
        whh1T = movingT(W_hh1, rows_g)

        # Wfold = Wih0[rows_g] @ W_o2: folds the x-term into gates0
        Wfold = W_ih0[rows_g] @ W_o2          # [G, 512]
        wfoldT = np.ascontiguousarray(
            Wfold.T.reshape(4, 128, G).transpose(1, 0, 2)).astype(np_dt)

        brows = np.zeros((1, 4, G), np.float32)
        brows[0, 0] = bias_g1[rows_g]
        brows[0, 1] = b_o1
        brows[0, 2] = bias_g0[rows_g] + W_ih0[rows_g] @ b_o2
        brows[0, 3] = bias_g0[rows_g]

        in_maps.append({
            "whh0T": whh0T, "wih1T": wih1T, "whh1T": whh1T,
            "wo1T": wo1T,
            "wfoldT": wfoldT,
            "wo2T": wo2T,
            "brows": brows.astype(np_dt),
            "wproj": wproj,
            "wprojcT": wprojcT,
            "zt": zt,
            "onesr": onesr,
            "ident": ident,
            "bo2": bo2_col,
        })
    return in_maps


def kernel(**inputs):
    from concourse.bass_utils import run_bass_kernel_spmd

    seq = int(os.environ.get("BASS_KERNEL_SEQ", SEQ))
    key = (seq, _mmdt())
    if key not in _CACHE:
        _CACHE[key] = _build(seq, _mmdt())
    nc = _CACHE[key]
    in_maps = _prep_inputs(inputs)

    trace = os.environ.get("BASS_KERNEL_TRACE", "") == "1"
    kwargs = {}
    if trace:
        kwargs["trace"] = True
        kwargs["tmpdir"] = os.environ.get("BASS_KERNEL_TRACE_DIR") or None
    res = run_bass_kernel_spmd(nc, in_maps, core_ids=list(range(N_CORES)),
                               **kwargs)
    if trace:
        kernel.last_exec_time_ns = res.exec_time_ns
    out = res.results[0]["out"]          # [seq, 8, B]
    return np.ascontiguousarray(out.transpose(2, 0, 1)).astype(np.float32)


kernel.last_exec_time_ns = None


# revision 36
# speedup vs baseline: 1.2153x; 1.0264x over previous
"""Trainium2 Bass kernel for a 2-layer LSTM decoder VAE head.

Strategy: 8-way tensor parallelism over the hidden dim (each core owns 128
rows of each gate / 512 gate rows per layer).  The key discovery from the
v1 trace: with weight-stationary matmuls the kernel is LDWEIGHTS-bound
(~117ns weight load per 53ns 128-wide stream).  So v2 flips the matmul
orientation: the gathered activation tiles (h1/h2, [feature,batch]) are the
*stationary* operand -- each loaded once per 2 weight streams -- and the
weights are the *moving* operand with free dim 512 (213ns streams that hide
the loads).  PSUM layout is batch-major [128b, 512gates]; biases and the
tiny Wih0@x term enter via rank-1/K=9 matmuls (ones-row trick).  The MLP
relu -> x path needs one layout flip, done with 4 small PE transposes per
batch-chunk.

B=256 runs as two 128-wide chunks (M=128 stationary limit) whose cycles
interleave so one chunk's AllGather hides behind the other's matmuls.
"""

import os
import numpy as np

B, SEQ, H, COORD = 256, 200, 1024, 8
LATS = (32, 64, 128)
TOT = sum(LATS)  # 224
N_CORES = 8
HC = H // N_CORES   # 128 rows of h per core
G = 4 * HC          # 512 gate rows per core
KT = H // 128       # 8 K tiles
NB = B // 2         # batch per chunk

_CACHE = {}


def _mmdt():
    return os.environ.get("BASS_KERNEL_MMDT", "bf16")


def _build(seq, mmdt):
    import concourse.bass as bass
    import concourse.tile as tile
    from concourse import bacc, mybir

    f32 = mybir.dt.float32
    DT = {"fp32": mybir.dt.float32, "bf16": mybir.dt.bfloat16}[mmdt]
    AF = mybir.ActivationFunctionType

    nc = bacc.Bacc("TRN2", target_bir_lowering=False, debug=False,
                   num_devices=N_CORES)

    def din(name, shape, dt=None):
        return nc.dram_tensor(name, list(shape), dt or f32,
                              kind="ExternalInput")

    # moving weights: [128 (k within tile), KT, 512 (gate cols)]
    whh0T = din("whh0T", (128, KT, G), DT)
    wih1T = din("wih1T", (128, KT, G), DT)
    whh1T = din("whh1T", (128, KT, G), DT)
    wo1T = din("wo1T", (128, KT, G), DT)       # MLP hidden (replicated)
    wfoldT = din("wfoldT", (128, 4, G), DT)    # (Wih0@Wo2)^T tiles
    wo2T = din("wo2T", (128, 4, COORD), DT)    # lhsT tiles for x
    brows = din("brows", (1, 4, G), DT)        # bias rows: [g1,o1,g0',g0]
    wproj = din("wproj", (128, KT, 2, 128))    # h0 lhsT tiles (f32)
    wprojcT = din("wprojcT", (128, 2, 128))    # c0 moving weights (f32)
    zt = din("zt", (128, 2, B))                # z^T padded, row 224 = ones
    onesr = din("onesr", (1, NB), DT)          # ones row at partition 0
    ident = din("ident", (128, 128), DT)
    bo2 = din("bo2", (COORD, 1))

    OUT = nc.dram_tensor("out", [seq, COORD, B], f32, kind="ExternalOutput")

    def persist(name, shape, dtype=f32):
        return nc.alloc_sbuf_tensor(name, list(shape), dtype).ap()

    whh0T_sb = persist("whh0T_sb", [128, KT, G], DT)
    wih1T_sb = persist("wih1T_sb", [128, KT, G], DT)
    whh1T_sb = persist("whh1T_sb", [128, KT, G], DT)
    wo1T_sb = persist("wo1T_sb", [128, KT, G], DT)
    wfoldT_sb = persist("wfoldT_sb", [128, 4, G], DT)
    wo2T_sb = persist("wo2T_sb", [128, 4, COORD], DT)
    brows_sb = persist("brows_sb", [1, 4, G], DT)
    wproj_sb = persist("wproj_sb", [128, KT, 2, 128])
    wprojcT_sb = persist("wprojcT_sb", [128, 2, 128])
    zt_sb = persist("zt_sb", [128, 2, B])
    ones_sb = persist("ones_sb", [1, NB], DT)
    ident_sb = persist("ident_sb", [128, 128], DT)
    bo2_sb = persist("bo2_sb", [COORD, 1])

    class Chain:
        def __init__(self, name, b0, dmaq):
            self.name = name
            self.b0 = b0
            self.dmaq = dmaq     # dedicated DMA queue engine for this chain
            self.h1T = persist(f"h1T_{name}", [128, KT, NB], DT)
            self.h2T = persist(f"h2T_{name}", [128, KT, NB], DT)
            self.c1 = persist(f"c1_{name}", [128, HC])   # batch-major f32
            self.c2 = persist(f"c2_{name}", [128, HC])
            self.reluT = persist(f"reluT_{name}", [128, 4, HC], DT)
            self.xTf = persist(f"xTf_{name}", [COORD, NB])
            self.g0 = None       # open gates0 psum (batch-major [128, G])
            self.g1_open = None  # g1 psum opened in front, closed in back
            self.pending = {}    # lay -> AllGather output dram tile

    with tile.TileContext(nc) as tc:
        A = Chain("a", 0, nc.sync)
        Bc = Chain("b", NB, nc.scalar)
        chains = (A, Bc)

        for dst, src in (
            (whh0T_sb, whh0T), (wih1T_sb, wih1T), (whh1T_sb, whh1T),
            (wo1T_sb, wo1T), (wfoldT_sb, wfoldT), (wo2T_sb, wo2T),
            (brows_sb, brows), (wproj_sb, wproj), (wprojcT_sb, wprojcT),
            (zt_sb, zt), (ones_sb, onesr), (ident_sb, ident),
            (bo2_sb, bo2),
        ):
            nc.sync.dma_start(dst[:], src.ap())

        with (
            tc.tile_pool(name="ps6", bufs=6, space="PSUM") as ps6,
            tc.tile_pool(name="psS", bufs=2, space="PSUM") as psS,
            tc.tile_pool(name="nl", bufs=4) as nl,
            tc.tile_pool(name="dram", bufs=2, space="DRAM") as dram,
        ):
            # ---- init: h0 (feature-major, replicated) ----
            for m in range(KT):
                ps = psS.tile([128, B], f32, name="ps_h0", tag="s")
                nc.tensor.matmul(ps[:], wproj_sb[:, m, 0, :], zt_sb[:, 0, :],
                                 start=True, stop=False)
                nc.tensor.matmul(ps[:], wproj_sb[:, m, 1, :], zt_sb[:, 1, :],
                                 start=False, stop=True)
                for ch in chains:
                    sl = ps[:, ch.b0:ch.b0 + NB]
                    nc.scalar.activation(ch.h1T[:, m, :], sl, AF.Identity)
                    nc.vector.tensor_copy(ch.h2T[:, m, :], ch.h1T[:, m, :])

            # ---- init: c0 (batch-major per chunk) ----
            for ch in chains:
                ps = psS.tile([128, HC], f32, name="ps_c0", tag="s")
                for kk in range(2):
                    nc.tensor.matmul(ps[:], zt_sb[:, kk, ch.b0:ch.b0 + NB],
                                     wprojcT_sb[:, kk, :],
                                     start=(kk == 0), stop=(kk == 1))
                nc.scalar.activation(ch.c1[:, :], ps[:], AF.Identity)
                nc.vector.tensor_copy(ch.c2[:, :], ch.c1[:, :])

            # ---- init: open gates0(0) = Whh0 @ h0 ----
            for ch in chains:
                g0 = ps6.tile([128, G], f32, name="ps_g0", tag="g")
                for k in range(KT):
                    nc.tensor.matmul(g0[:], ch.h1T[:, k, :], whh0T_sb[:, k, :],
                                     start=(k == 0), stop=False)
                ch.g0 = g0

            def gather(ch, src_sb, lay):
                """Launch AllGather of the transposed h chunk (SBUF
                [128f, NB]).  The scatter back to SBUF is emitted separately
                at the consumption point (scatter())."""
                sfx = f"{ch.name}{lay}"
                inb = dram.tile([128, NB], DT, name="agin", tag=f"agi_{sfx}")
                outb = dram.tile([128 * N_CORES, NB], DT, name="agout",
                                 tag=f"ago_{sfx}")
                nc.scalar.dma_start(inb[:], src_sb[:])
                nc.gpsimd.collective_compute(
                    "AllGather", mybir.AluOpType.bypass,
                    replica_groups=[list(range(N_CORES))],
                    ins=[inb.opt()], outs=[outb.opt()],
                )
                ch.pending[lay] = outb

            def scatter(ch, lay, dest):
                outb = ch.pending[lay]
                nc.sync.dma_start(
                    dest[:, :, :],
                    outb[:, :].rearrange("(k p) n -> p k n", p=128))

            def nonlin(ch, gps, c_sb, lay):
                """gps: batch-major psum [128b, 512] = [i|f|g|o].
                Returns h chunk [128b, 128f] DT in SBUF."""
                sfx = f"{ch.name}{lay}"
                sig_if = nl.tile([128, 2 * HC], f32, name="sig_if",
                                 tag=f"si_{sfx}")
                tan_g = nl.tile([128, HC], f32, name="tan_g", tag=f"tg_{sfx}")
                sig_o = nl.tile([128, HC], f32, name="sig_o", tag=f"so_{sfx}")
                nc.scalar.activation(sig_if[:], gps[:, 0:2 * HC], AF.Sigmoid)
                nc.scalar.activation(tan_g[:], gps[:, 2 * HC:3 * HC], AF.Tanh)
                nc.scalar.activation(sig_o[:], gps[:, 3 * HC:4 * HC],
                                     AF.Sigmoid)
                t_fc = nl.tile([128, HC], f32, name="t_fc", tag=f"fc_{sfx}")
                t_ig = nl.tile([128, HC], f32, name="t_ig", tag=f"ig_{sfx}")
                nc.vector.tensor_mul(t_fc[:], sig_if[:, HC:2 * HC], c_sb[:, :])
                nc.vector.tensor_mul(t_ig[:], sig_if[:, 0:HC], tan_g[:])
                nc.vector.tensor_add(c_sb[:, :], t_fc[:], t_ig[:])
                tan_c = nl.tile([128, HC], f32, name="tan_c", tag=f"tc_{sfx}")
                nc.scalar.activation(tan_c[:], c_sb[:, :], AF.Tanh)
                hch = nl.tile([128, HC], DT, name="hch", tag=f"h_{sfx}")
                nc.vector.tensor_mul(hch[:], sig_o[:], tan_c[:])
                return hch

            def transpose_gather(ch, hch, lay):
                tp = psS.tile([128, NB], DT, name="ps_tr", tag="s")
                nc.tensor.transpose(tp[:], hch[:], ident_sb[:])
                hT = nl.tile([128, NB], DT, name="hT",
                             tag=f"hT_{ch.name}{lay}")
                nc.vector.tensor_copy(hT[:], tp[:])
                gather(ch, hT, lay)

            def emit_front(ch, t):
                """Needs h2T = h2(t-1) gathered.  h2-stationary matmul block;
                (t>0) relu(t-1) + transposes + the Wfold term completing
                gates0(t); nonlin0 -> h1(t); launch AG1(t); then the
                off-cycle x(t-1) -> OUT store (fills the AG wait)."""
                if t > 0:
                    scatter(ch, 1, ch.h2T)   # AG2(t-1) lands here
                    # critical path: MLP -> relu -> reluT -> close gates0(t)
                    mlp = ps6.tile([128, G], f32, name="ps_mlp", tag="g")
                    for k in range(KT):
                        nc.tensor.matmul(mlp[:], ch.h2T[:, k, :],
                                         wo1T_sb[:, k, :],
                                         start=(k == 0), stop=False)
                    nc.tensor.matmul(mlp[:], ones_sb[:], brows_sb[0:1, 1, :],
                                     start=False, stop=True)
                    relu = nl.tile([128, G], DT, name="relu",
                                   tag=f"relu_{ch.name}")
                    nc.scalar.activation(relu[:], mlp[:], AF.Relu)
                    for j in range(4):
                        tp = psS.tile([128, HC], DT, name="ps_rT", tag="s")
                        nc.tensor.transpose(
                            tp[:], relu[:, j * HC:(j + 1) * HC], ident_sb[:])
                        nc.vector.tensor_copy(ch.reluT[:, j, :], tp[:])
                    # gates0(t) += Wfold @ relu(t-1) + bias' (folds Wih0@x)
                    nc.tensor.matmul(ch.g0[:], ones_sb[:],
                                     brows_sb[0:1, 2, :],
                                     start=False, stop=False)
                    for j in range(4):
                        nc.tensor.matmul(ch.g0[:], ch.reluT[:, j, :],
                                         wfoldT_sb[:, j, :],
                                         start=False, stop=(j == 3))
                else:
                    nc.tensor.matmul(ch.g0[:], ones_sb[:],
                                     brows_sb[0:1, 3, :],
                                     start=False, stop=True)
                h1ch = nonlin(ch, ch.g0, ch.c1, 0)
                transpose_gather(ch, h1ch, 0)
                # ---- AG1(t) in flight: fill with non-critical PE work ----
                g1 = ps6.tile([128, G], f32, name="ps_g1", tag="g")
                ch.g1_open = g1
                for k in range(KT):
                    nc.tensor.matmul(g1[:], ch.h2T[:, k, :],
                                     whh1T_sb[:, k, :],
                                     start=(k == 0), stop=False)
                nc.tensor.matmul(g1[:], ones_sb[:], brows_sb[0:1, 0, :],
                                 start=False, stop=False)
                if t > 0:
                    # off-cycle: x(t-1) for the OUT store
                    psx = psS.tile([COORD, NB], f32, name="ps_x", tag="s")
                    for j in range(4):
                        nc.tensor.matmul(psx[:], wo2T_sb[:, j, :],
                                         ch.reluT[:, j, :],
                                         start=(j == 0), stop=(j == 3))
                    nc.scalar.activation(ch.xTf[:, :], psx[:],
                                         AF.Identity, bias=bo2_sb[:, 0:1])
                    nc.scalar.dma_start(OUT.ap()[t - 1][:, ch.b0:ch.b0 + NB],
                                        ch.xTf[:, :])

            def emit_back(ch, t):
                """Needs h1T = h1(t) gathered.  Completes gates1(t) (Wih1
                part), opens gates0(t+1) (Whh0 part); nonlin1 -> h2(t);
                launches AG2(t)."""
                scatter(ch, 0, ch.h1T)   # AG1(t) lands here
                last = t == seq - 1
                g1 = ch.g1_open
                for k in range(KT):
                    nc.tensor.matmul(g1[:], ch.h1T[:, k, :],
                                     wih1T_sb[:, k, :],
                                     start=False, stop=(k == KT - 1))
                h2ch = nonlin(ch, g1, ch.c2, 1)
                transpose_gather(ch, h2ch, 1)
                # ---- AG2(t) in flight: open gates0(t+1) (Whh0 part) ----
                g0n = None
                if not last:
                    g0n = ps6.tile([128, G], f32, name="ps_g0", tag="g")
                    for k in range(KT):
                        nc.tensor.matmul(g0n[:], ch.h1T[:, k, :],
                                         whh0T_sb[:, k, :],
                                         start=(k == 0), stop=False)
                ch.g0 = g0n

            def emit_tail(ch):
                """x(seq-1) from gathered h2(seq-1): MLP only, store OUT."""
                scatter(ch, 1, ch.h2T)   # AG2(seq-1) lands here
                mlp = ps6.tile([128, G], f32, name="ps_mlp", tag="g")
                for k in range(KT):
                    nc.tensor.matmul(mlp[:], ch.h2T[:, k, :],
                                     wo1T_sb[:, k, :],
                                     start=(k == 0), stop=False)
                nc.tensor.matmul(mlp[:], ones_sb[:], brows_sb[0:1, 1, :],
                                 start=False, stop=True)
                relu = nl.tile([128, G], DT, name="relu",
                               tag=f"relu_{ch.name}")
                nc.scalar.activation(relu[:], mlp[:], AF.Relu)
                for j in range(4):
                    tp = psS.tile([128, HC], DT, name="ps_rT", tag="s")
                    nc.tensor.transpose(
                        tp[:], relu[:, j * HC:(j + 1) * HC], ident_sb[:])
                    nc.vector.tensor_copy(ch.reluT[:, j, :], tp[:])
                psx = psS.tile([COORD, NB], f32, name="ps_x", tag="s")
                for j in range(4):
                    nc.tensor.matmul(psx[:], wo2T_sb[:, j, :],
                                     ch.reluT[:, j, :],
                                     start=(j == 0), stop=(j == 3))
                nc.scalar.activation(ch.xTf[:, :], psx[:], AF.Identity,
                                     bias=bo2_sb[:, 0:1])
                nc.scalar.dma_start(OUT.ap()[seq - 1][:, ch.b0:ch.b0 + NB],
                                    ch.xTf[:, :])

            for t in range(seq):
                for ch in chains:
                    emit_front(ch, t)
                for ch in chains:
                    emit_back(ch, t)
            for ch in chains:
                emit_tail(ch)

    nc.compile()
    return nc


def _prep_inputs(inputs):
    import ml_dtypes
    np_dt = {"fp32": np.float32, "bf16": ml_dtypes.bfloat16}[_mmdt()]
    f = lambda k: np.asarray(inputs[k], np.float32)
    W_proj, b_proj = f("W_proj"), f("b_proj")
    W_ih0, W_hh0 = f("W_ih0"), f("W_hh0")
    b_ih0, b_hh0 = f("b_ih0"), f("b_hh0")
    W_ih1, W_hh1 = f("W_ih1"), f("W_hh1")
    b_ih1, b_hh1 = f("b_ih1"), f("b_hh1")
    W_o1, b_o1 = f("W_o1"), f("b_o1")
    W_o2, b_o2 = f("W_o2"), f("b_o2")
    z = np.concatenate([f("z_primitive"), f("z_skill"), f("z_style")], axis=1)

    bias_g0 = b_ih0 + b_hh0
    bias_g1 = b_ih1 + b_hh1

    # z^T padded to 256 rows, row 224 = ones (bias row for the init matmuls)
    ztp = np.zeros((256, B), np.float32)
    ztp[:TOT] = z.T
    ztp[TOT] = 1.0
    zt = np.ascontiguousarray(ztp.reshape(2, 128, B).transpose(1, 0, 2))

    ident = np.eye(128, dtype=np.float32).astype(np_dt)
    onesr = np.ones((1, NB), np.float32).astype(np_dt)

    def movingT(W, rows):
        """W[rows] -> moving rhs tiles [128, KT', len(rows)]:
        out[p, k, n] = W[rows[n], k*128+p]."""
        Wt = np.ascontiguousarray(W[rows].T)  # [K, G]
        K = Wt.shape[0]
        return np.ascontiguousarray(
            Wt.reshape(K // 128, 128, len(rows)).transpose(1, 0, 2)
        ).astype(np_dt)

    # MLP hidden weights (replicated): K=1024 over h2, N=512 hidden
    wo1T = movingT(W_o1, np.arange(H // 2))
    # x lhsT tiles: wo2T[p, j, m] = W_o2[m, j*128+p]
    wo2T = np.ascontiguousarray(
        W_o2.T.reshape(4, 128, COORD).transpose(1, 0, 2)).astype(np_dt)
    bo2_col = b_o2.reshape(COORD, 1).astype(np.float32)

    # h0 lhsT tiles (f32): wproj[p, m, kk, n] = Wp[m*128+n, kk*128+p]
    Wp = np.zeros((H, 256), np.float32)
    Wp[:, :TOT] = W_proj[:H]
    Wp[:, TOT] = b_proj[:H]
    wproj = np.ascontiguousarray(
        Wp.reshape(KT, 128, 2, 128).transpose(3, 0, 2, 1))

    in_maps = []
    for c in range(N_CORES):
        rows_g = np.concatenate(
            [g * H + c * HC + np.arange(HC) for g in range(4)])
        # c0 moving weights: wprojcT[p, kk, n] = Wpc[n, kk*128+p]
        own = H + c * HC + np.arange(HC)
        Wpc = np.zeros((HC, 256), np.float32)
        Wpc[:, :TOT] = W_proj[own]
        Wpc[:, TOT] = b_proj[own]
        wprojcT = np.ascontiguousarray(
            Wpc.T.reshape(2, 128, HC).transpose(1, 0, 2))

        whh0T = movingT(W_hh0, rows_g)
        wih1T = movingT(W_ih1, rows_g)
        whh1T = movingT(W_hh1, rows_g)

        # Wfold = Wih0[rows_g] @ W_o2: folds the x-term into gates0
        Wfold = W_ih0[rows_g] @ W_o2          # [G, 512]
        wfoldT = np.ascontiguousarray(
            Wfold.T.reshape(4, 128, G).transpose(1, 0, 2)).astype(np_dt)

        brows = np.zeros((1, 4, G), np.float32)
        brows[0, 0] = bias_g1[rows_g]
        brows[0, 1] = b_o1
        brows[0, 2] = bias_g0[rows_g] + W_ih0[rows_g] @ b_o2
        brows[0, 3] = bias_g0[rows_g]

        in_maps.append({
            "whh0T": whh0T, "wih1T": wih1T, "whh1T": whh1T,
            "wo1T": wo1T,
            "wfoldT": wfoldT,
            "wo2T": wo2T,
            "brows": brows.astype(np_dt),
            "wproj": wproj,
            "wprojcT": wprojcT,
            "zt": zt,
            "onesr": onesr,
            "ident": ident,
            "bo2": bo2_col,
        })
    return in_maps


def kernel(**inputs):
    from concourse.bass_utils import run_bass_kernel_spmd

    seq = int(os.environ.get("BASS_KERNEL_SEQ", SEQ))
    key = (seq, _mmdt())
    if key not in _CACHE:
        _CACHE[key] = _build(seq, _mmdt())
    nc = _CACHE[key]
    in_maps = _prep_inputs(inputs)

    trace = os.environ.get("BASS_KERNEL_TRACE", "") == "1"
    kwargs = {}
    if trace:
        kwargs["trace"] = True
        kwargs["tmpdir"] = os.environ.get("BASS_KERNEL_TRACE_DIR") or None
    res = run_bass_kernel_spmd(nc, in_maps, core_ids=list(range(N_CORES)),
                               **kwargs)
    if trace:
        kernel.last_exec_time_ns = res.exec_time_ns
    out = res.results[0]["out"]          # [seq, 8, B]
    return np.ascontiguousarray(out.transpose(2, 0, 1)).astype(np.float32)


kernel.last_exec_time_ns = None
